# revision 22
# baseline (speedup 1.0000x reference)
"""GQA MultiHeadAttention (RoPE, causal) Bass/Tile kernel for 8 Trainium2 cores.

Problem: x[2,2048,2048] @ Wq/Wk/Wv -> RoPE -> causal GQA attention -> @ Wo.
D=2048, H=16 heads, G=4 KV groups, HD=128, B=2, S=2048.

Sharding (SPMD, one program, per-core data):
  core c -> batch b=c//4, KV-group g=c%4 (heads 4g..4g+3).
  Each core: QKV projection for its group from x[b]^T, RoPE, 4 heads of
  attention, and a row-shard of the output projection (Wo rows for its
  heads) producing a partial [2048,2048] output. Host sums the 4 partials
  per batch.

Design notes (vs v1 baseline at ~367us):
  - bf16 matmul path throughout (x, Wq/k/v/o, q/k, exp-weights, V, ctx);
    fp32 PSUM accumulation. Tolerance is 2e-2; this lands ~3.6e-3.
  - softmax denominator: no per-tile PE matmuls; exp tiles accumulate into
    an SBUF esum on DVE (bf16 2x), then ONE ones-vector matmul per
    (head, q-chunk) reduces over partitions.
  - causal trimming: diagonal k-tiles only compute scores/ctx/den on the
    q-column range at-or-below the diagonal; masks (widened to also clear
    the never-written columns) run on GpSimd.
  - k-tiles processed in PAIRS sharing a 2-bank PSUM tile so one ACT exp
    covers both (amortizes ~400ns/op ACT overhead). ACT does only exp in
    the attention phase; evictions are split across DVE/ACT.
  - DMA batching: descriptor issue costs ~0.6us on the Sync engine, so x
    and weights load in multi-d-tile groups, Wo is SBUF-resident, and the
    out-projection stores whole [128,2048] rows.
  - PSUM plan (8 banks): 2x 2-bank pair tiles (QKV q-accumulators /
    scores pairs) + 2 single banks (K/V accumulators / ctx + den-reduce)
    + 2 single banks (rope-rotation + V-transpose / broadcast + outproj).
  - rope/V-transpose tails interleave into the next s-chunk's projection
    stream; each q-chunk's out-projection interleaves into the next
    q-chunk's attention stream.
On-chip layouts all keep the feature dim on partitions: QT/KT [hd, s],
scoresT [k, q], ctxT [hd, q]; out-projection uses ctxT slices as stationary
to produce natural [s, d] partials.
"""

import sys

if "/opt/trn_rl_repo" not in sys.path:
    sys.path.insert(0, "/opt/trn_rl_repo")

from contextlib import ExitStack

import numpy as np
import ml_dtypes

import concourse.bass as bass
import concourse.tile as tile
from concourse import bacc, mybir
from concourse.bass_utils import run_bass_kernel_spmd
from concourse.masks import make_identity

F32 = mybir.dt.float32
F32R = mybir.dt.float32r
BF16 = mybir.dt.bfloat16
AF = mybir.ActivationFunctionType
BF16NP = ml_dtypes.bfloat16

B, S, D = 2, 2048, 2048
H, G, HD = 16, 4, 128
HPG = H // G          # heads per group = 4
GD = HPG * HD         # group width = 512
P = 128
NCHUNK = 512          # matmul moving free dim
SC = S // NCHUNK      # 4 s-chunks
DT = D // P           # 16 d-tiles
ST = S // P           # 16 s-tiles
SCALE = 1.0 / float(np.sqrt(HD))

_CACHE = {}


def _build():
    nc = bacc.Bacc("TRN2", target_bir_lowering=False, debug=False, num_devices=8)

    # ---- DRAM I/O (per-core shards) ----
    xT = nc.dram_tensor("xT", [D, S], BF16, kind="ExternalInput").ap()
    wq = nc.dram_tensor("wq", [D, GD], BF16, kind="ExternalInput").ap()
    wk = nc.dram_tensor("wk", [D, HD], BF16, kind="ExternalInput").ap()
    wv = nc.dram_tensor("wv", [D, HD], BF16, kind="ExternalInput").ap()
    wo = nc.dram_tensor("wo", [GD, D], BF16, kind="ExternalInput").ap()
    cosT = nc.dram_tensor("cosT", [HD, S], BF16, kind="ExternalInput").ap()
    sinT = nc.dram_tensor("sinT", [HD, S], BF16, kind="ExternalInput").ap()
    prot = nc.dram_tensor("prot", [HD, HD], BF16, kind="ExternalInput").ap()
    onesc = nc.dram_tensor("onesc", [P, 1], BF16, kind="ExternalInput").ap()
    onesr = nc.dram_tensor("onesr", [1, P], F32R, kind="ExternalInput").ap()
    out = nc.dram_tensor("out", [S, D], F32, kind="ExternalOutput").ap()

    xT_v = xT.rearrange("(t p) s -> p t s", p=P)          # [128, 16, 2048]
    wq_v = wq.rearrange("(t p) o -> p t o", p=P)          # [128, 16, 512]
    wk_v = wk.rearrange("(t p) o -> p t o", p=P)          # [128, 16, 128]
    wv_v = wv.rearrange("(t p) o -> p t o", p=P)
    wo_v = wo.rearrange("(h p) d -> p h d", p=P)          # [128, 4, 2048]
    out_v = out.rearrange("(t p) d -> t p d", p=P)        # [16, 128, 2048]

    with tile.TileContext(nc) as tc:
        with ExitStack() as ctx:
            pers = ctx.enter_context(tc.tile_pool(name="pers", bufs=1))
            psum = ctx.enter_context(tc.tile_pool(name="psum", bufs=2, space="PSUM"))
            rpsum = ctx.enter_context(tc.tile_pool(name="rpsum", bufs=2, space="PSUM"))
            xpool = ctx.enter_context(tc.tile_pool(name="xpool", bufs=2))
            x8pool = ctx.enter_context(tc.tile_pool(name="x8pool", bufs=3))
            spool = ctx.enter_context(tc.tile_pool(name="spool", bufs=3))
            epool = ctx.enter_context(tc.tile_pool(name="epool", bufs=4))
            dpool = ctx.enter_context(tc.tile_pool(name="dpool", bufs=3))
            cpool = ctx.enter_context(tc.tile_pool(name="cpool", bufs=2))
            evpool = ctx.enter_context(tc.tile_pool(name="evpool", bufs=2))
            opool = ctx.enter_context(tc.tile_pool(name="opool", bufs=2))

            _bank_n = [0]

            def bank():
                _bank_n[0] += 1
                return psum.tile([P, NCHUNK], F32, tag="bank",
                                 name=f"bank{_bank_n[0]}")

            def qpair():
                _bank_n[0] += 1
                return psum.tile([P, 2, NCHUNK], F32, tag="qpair",
                                 name=f"qpair{_bank_n[0]}")

            def rbank():
                _bank_n[0] += 1
                return rpsum.tile([P, NCHUNK], F32, tag="rbank",
                                  name=f"rbank{_bank_n[0]}")

            # ---- persistent tiles ----
            wq_t = pers.tile([P, DT, GD], BF16, tag="wq")
            wk_t = pers.tile([P, DT, HD], BF16, tag="wk")
            wv_t = pers.tile([P, DT, HD], BF16, tag="wv")
            cos_t = pers.tile([P, S], BF16, tag="cos")
            sin_t = pers.tile([P, S], BF16, tag="sin")
            prot_t = pers.tile([P, HD], BF16, tag="prot")
            ident = pers.tile([P, P], F32, tag="ident")
            qf = pers.tile([P, HPG, S], BF16, tag="qf")       # roped Q^T, 4 heads
            kf = pers.tile([P, S], BF16, tag="kf")            # roped K^T
            vnat = pers.tile([P, ST, HD], BF16, tag="vnat")   # V natural [s, hd]
            ones_col = pers.tile([P, 1], BF16, tag="ones_col")    # [K=128, M=1]
            ones_row = pers.tile([1, P], F32R, tag="ones_row")    # [K=1, M=128]
            wo_t = pers.tile([P, HPG, D], BF16, tag="wo")         # Wo rows, resident

            # Batched loads: DMA descriptor issue costs ~0.6us each on the
            # Sync queue, so group x/weight tiles into few large transfers.
            # sc0's x comes in 4-dt groups so the first matmuls start early.
            groups0 = [(0, 2), (2, 4), (4, 8), (8, 12), (12, 16)]
            x_tiles = {0: []}
            for lo, hi in groups0:
                nc.sync.dma_start(wq_t[:, lo:hi, :], wq_v[:, lo:hi, :])
                xg = xpool.tile([P, hi - lo, NCHUNK], BF16,
                                tag=f"xg{hi - lo}", name=f"xg0_{lo}")
                nc.sync.dma_start(xg[:], xT_v[:, lo:hi, 0:NCHUNK])
                x_tiles[0].append((lo, hi, xg))
            nc.sync.dma_start(wk_t[:], wk_v[:])
            nc.sync.dma_start(wv_t[:], wv_v[:])
            nc.sync.dma_start(cos_t[:], cosT[:])
            nc.sync.dma_start(sin_t[:], sinT[:])
            nc.sync.dma_start(prot_t[:], prot[:])
            nc.sync.dma_start(ones_col[:], onesc[:])
            nc.sync.dma_start(ones_row[:], onesr[:])
            make_identity(nc, ident[:])

            def fetch_x8(sc, g):
                xg = x8pool.tile([P, 8, NCHUNK], BF16, tag="xg8",
                                 name=f"xg{sc}_{g}")
                nc.sync.dma_start(
                    xg[:], xT_v[:, 8 * g:8 * (g + 1),
                                sc * NCHUNK:(sc + 1) * NCHUNK])
                return xg

            # ================= Phase A: QKV projection + RoPE + V^T -> V ====
            def rope(dst, src_sb, sc):
                """dst (bf16 slice [128,512]) = rope(src_sb bf16) for s-chunk sc."""
                cs = cos_t[:, sc * NCHUNK:(sc + 1) * NCHUNK]
                sn = sin_t[:, sc * NCHUNK:(sc + 1) * NCHUNK]
                rotps = rbank()
                nc.tensor.matmul(rotps[:], prot_t[:], src_sb, start=True, stop=True)
                t1 = spool.tile([P, NCHUNK], F32, tag="t1")
                nc.vector.tensor_mul(t1[:], rotps[:], sn)
                t2 = spool.tile([P, NCHUNK], F32, tag="t2")
                nc.vector.tensor_mul(t2[:], src_sb, cs)
                nc.vector.tensor_add(dst, t1[:], t2[:])

            def phase_a_proj(sc, dts):
                """QKV projection matmuls for s-chunk sc over d-tiles dts."""
                for dt in dts:
                    if sc == 0:
                        for lo, hi, xg in x_tiles[0]:
                            if lo <= dt < hi:
                                xt = xg[:, dt - lo, :]
                                break
                    else:
                        xt = x_tiles[sc][dt // 8][:, dt % 8, :]
                    st_flag = dt == 0
                    sp_flag = dt == DT - 1
                    qpr, kps, vps = _acc[sc]
                    for h in range(HPG):
                        nc.tensor.matmul(
                            qpr[h // 2][:, h % 2, :],
                            wq_t[:, dt, h * HD:(h + 1) * HD], xt,
                            start=st_flag, stop=sp_flag)
                    nc.tensor.matmul(kps[:], wk_t[:, dt, :], xt,
                                     start=st_flag, stop=sp_flag)
                    nc.tensor.matmul(vps[:], wv_t[:, dt, :], xt,
                                     start=st_flag, stop=sp_flag)
                    if sc + 1 < SC and dt in (6, 10):
                        x_tiles.setdefault(sc + 1, []).append(
                            fetch_x8(sc + 1, (dt - 6) // 4))

            def phase_a_evict(sc):
                """PSUM -> SBUF bf16 evictions (frees the 6 accumulator banks)."""
                qpr, kps, vps = _acc[sc]
                qsb4 = evpool.tile([P, HPG, NCHUNK], BF16, tag="ev_q",
                                   name=f"qsb{sc}")
                for h in range(HPG):
                    if h < 2:
                        nc.scalar.copy(qsb4[:, h, :], qpr[h // 2][:, h % 2, :])
                    else:
                        nc.vector.tensor_copy(qsb4[:, h, :],
                                              qpr[h // 2][:, h % 2, :])
                ksb = evpool.tile([P, NCHUNK], BF16, tag="ev_k", name=f"ksb{sc}")
                nc.vector.tensor_copy(ksb[:], kps[:])
                vsb = evpool.tile([P, NCHUNK], F32, tag="ev_v", name=f"vsb{sc}")
                nc.scalar.copy(vsb[:], vps[:])
                return qsb4, ksb, vsb

            def phase_a_tail_steps(sc, qsb4, ksb, vsb):
                """RoPE + V^T transpose (PE + DVE work, consumes evictions).
                Generator so the last s-chunk's tail can interleave with the
                first q-chunk's attention (fully data-independent)."""
                s0 = sc * NCHUNK
                for h in range(HPG):
                    rope(qf[:, h, s0:s0 + NCHUNK], qsb4[:, h, :], sc)
                    yield
                rope(kf[:, s0:s0 + NCHUNK], ksb[:], sc)
                yield
                tb = rbank()
                for j in range(4):
                    nc.tensor.transpose(
                        tb[:, j * P:(j + 1) * P], vsb[:, j * P:(j + 1) * P],
                        ident[:])
                yield
                nc.vector.tensor_copy(vnat[:, sc * 4:(sc + 1) * 4, :], tb[:])
                nc.sync.dma_start(wo_t[:, :, sc * NCHUNK:(sc + 1) * NCHUNK],
                                  wo_v[:, :, sc * NCHUNK:(sc + 1) * NCHUNK])
                yield

            def phase_a_tail(sc, qsb4, ksb, vsb):
                for _ in phase_a_tail_steps(sc, qsb4, ksb, vsb):
                    pass

            # ---- Phase B/C: attention per (q-chunk, head) + out-projection
            def den_engine(h):
                return nc.vector

            def norm_apply(ctxq, h, ctxps, denps):
                """reciprocal + partition-broadcast (K=1 matmul) + scale ctx."""
                rec32 = spool.tile([1, NCHUNK], F32, tag="rec32")
                nc.vector.reciprocal_approx_fast(rec32[:], denps[0:1, :])
                rec = spool.tile([1, NCHUNK], F32R, tag="rec")
                nc.vector.tensor_copy(rec[:], rec32[:])
                bps = rbank()
                nc.tensor.matmul(bps[:], ones_row[:], rec[:],
                                 start=True, stop=True)
                bsb = spool.tile([P, NCHUNK], F32, tag="bsb")
                nc.scalar.copy(bsb[:], bps[:])
                nc.vector.tensor_mul(ctxq[:, h, :], ctxps[:], bsb[:])

            LOOKAHEAD = 2   # in pairs (~4 k-tiles)

            def attention_steps(qc, ctxq):
                """Generator emitting attention for q-chunk qc one pipeline
                step at a time. k-tiles are processed in PAIRS sharing a
                2-bank PSUM tile so one ACT exp covers both (amortizes the
                ~400ns per-op ACT overhead). Scores matmuls stay causally
                trimmed; the diagonal masks are widened to also zero the
                never-written (garbage) PSUM columns of trimmed tiles."""
                q0 = qc * NCHUNK
                nki = 4 * qc + 4
                order = list(range(4 * qc, nki)) + list(range(4 * qc))
                npair = nki // 2
                pairs_seq = [(h, pi) for h in range(HPG) for pi in range(npair)]
                ets = {}
                esums = {}
                accs = {}
                pending = [None]

                def qlo_of(ki):
                    j = ki - 4 * qc
                    return P * j if j >= 0 else 0

                def do_scores(h, pi):
                    ki0, ki1 = order[2 * pi], order[2 * pi + 1]
                    sps = qpair()
                    for sl, ki in ((0, ki0), (1, ki1)):
                        qlo = qlo_of(ki)
                        nc.tensor.matmul(
                            sps[:, sl, qlo:], kf[:, ki * P:(ki + 1) * P],
                            qf[:, h, q0 + qlo:q0 + NCHUNK],
                            start=True, stop=True)
                    et2 = epool.tile([P, 2, NCHUNK], BF16, tag="et2",
                                     name=f"et{qc}_{h}_{pi}")
                    nc.scalar.activation(et2[:], sps[:], AF.Exp, scale=SCALE)
                    for sl, ki in ((0, ki0), (1, ki1)):
                        j = ki - 4 * qc
                        if j >= 0:
                            # zero below-diagonal AND the garbage columns
                            # left of the trimmed range: cols [0, 128(j+1))
                            nc.gpsimd.affine_select(
                                out=et2[:, sl, 0:P * (j + 1)],
                                in_=et2[:, sl, 0:P * (j + 1)],
                                compare_op=mybir.AluOpType.is_ge,
                                fill=0.0,
                                base=-(P * j),
                                channel_multiplier=-1,
                                pattern=[[1, P * (j + 1)]],
                            )
                    for sl, ki in ((0, ki0), (1, ki1)):
                        qlo = qlo_of(ki)
                        if pi == 0 and sl == 0:
                            esum = dpool.tile([P, NCHUNK], BF16, tag="esum",
                                              name=f"esum{qc}_{h}")
                            esums[h] = esum
                            nc.vector.tensor_copy(esum[:], et2[:, 0, :])
                        else:
                            esum = esums[h]
                            nc.vector.tensor_add(esum[:, qlo:], esum[:, qlo:],
                                                 et2[:, sl, qlo:])
                    ets[(h, pi)] = et2

                def do_ctx(h, pi):
                    if h not in accs:
                        accs[h] = bank()
                        if pending[0] is not None:
                            norm_apply(ctxq, *pending[0])
                            pending[0] = None
                    ctxps = accs[h]
                    ki0, ki1 = order[2 * pi], order[2 * pi + 1]
                    et2 = ets.pop((h, pi))
                    for sl, ki in ((0, ki0), (1, ki1)):
                        qlo = qlo_of(ki)
                        nc.tensor.matmul(
                            ctxps[:, qlo:], vnat[:, ki, :], et2[:, sl, qlo:],
                            start=(pi == 0 and sl == 0),
                            stop=(pi == npair - 1 and sl == 1))
                    if pi == npair - 1:
                        # head finished: one partition-reduce matmul for the
                        # denominator; recip+broadcast+apply deferred into
                        # the next head's stream
                        denps = rbank()
                        nc.tensor.matmul(denps[0:1, :], ones_col[:],
                                         esums.pop(h)[:], start=True,
                                         stop=True)
                        pending[0] = (h, ctxps, denps)
                        del accs[h]

                for i, (h, pi) in enumerate(pairs_seq):
                    do_scores(h, pi)
                    if i >= LOOKAHEAD:
                        do_ctx(*pairs_seq[i - LOOKAHEAD])
                    yield
                for i in range(len(pairs_seq) - LOOKAHEAD, len(pairs_seq)):
                    do_ctx(*pairs_seq[i])
                    yield
                norm_apply(ctxq, *pending[0])

            def outproj_steps(qc, ctxq):
                """Generator emitting the out-projection for q-chunk qc one
                (dc, st) group at a time; interleaved into the NEXT q-chunk's
                attention stream so its Wo DMAs and eviction chains hide."""
                for st in range(4):
                    stq = qc * 4 + st
                    osb = opool.tile([P, SC, NCHUNK], F32, tag="osb",
                                     name=f"osb{qc}_{st}")
                    for dc in range(SC):
                        ops = rbank()
                        for h in range(HPG):
                            nc.tensor.matmul(
                                ops[:], ctxq[:, h, st * P:(st + 1) * P],
                                wo_t[:, h, dc * NCHUNK:(dc + 1) * NCHUNK],
                                start=(h == 0), stop=(h == HPG - 1))
                        if dc % 2 == 0:
                            nc.vector.tensor_copy(osb[:, dc, :], ops[:])
                        else:
                            nc.scalar.copy(osb[:, dc, :], ops[:])
                        yield
                    nc.sync.dma_start(out_v[stq, :, :], osb[:])

            # Interleave: s-chunk tails (rope/transpose, PE+DVE) are emitted
            # after the next s-chunk's first projection matmuls so the PE
            # queue never drains while evictions/ropes complete.
            _acc = {}
            tail_gen = None
            for sc in range(SC):
                _acc[sc] = ([qpair(), qpair()], bank(), bank())
                for dt in range(DT):
                    phase_a_proj(sc, [dt])
                    if tail_gen is not None and dt >= 5 and dt % 2 == 1:
                        next(tail_gen, None)
                if tail_gen is not None:
                    for _ in tail_gen:
                        pass
                tail_gen = phase_a_tail_steps(sc, *phase_a_evict(sc))

            # Phase B: each q-chunk's attention, with the PREVIOUS q-chunk's
            # out-projection groups spread through it. The final s-chunk's
            # rope/transpose tail seeds the interleave stream for q-chunk 0.
            out_gen = tail_gen
            for qc in range(SC):
                ctxq = cpool.tile([P, HPG, NCHUNK], BF16, tag="ctxq",
                                  name=f"ctxq{qc}")
                n_steps = 4 * (2 * qc + 2) + LOOKAHEAD
                ratio = max(1, n_steps // 17)
                k = 0
                delay = 6 if qc == 0 else 0
                for _ in attention_steps(qc, ctxq):
                    k += 1
                    if out_gen is not None and k >= delay and k % ratio == 0:
                        next(out_gen, None)
                if out_gen is not None:
                    for _ in out_gen:
                        pass
                out_gen = outproj_steps(qc, ctxq)
            for _ in out_gen:
                pass

    nc.compile()
    return nc


def _host_consts():
    i = np.arange(0, HD, 2, dtype=np.float32)
    inv = (1.0 / (10000.0 ** (i / HD))).astype(np.float32)      # [64]
    t = np.arange(S, dtype=np.float32)
    freqs = t[:, None] * inv[None, :]                           # [S, 64] f32
    emb = np.concatenate([freqs, freqs], axis=1)                # [S, 128]
    cosT = np.ascontiguousarray(np.cos(emb).T.astype(BF16NP))   # [128, S]
    sinT = np.ascontiguousarray(np.sin(emb).T.astype(BF16NP))
    prot = np.zeros((HD, HD), dtype=np.float32)
    half = HD // 2
    for ii in range(half):
        prot[ii + half, ii] = -1.0     # rot[i] = -x[i+64], i < 64
    for ii in range(half, HD):
        prot[ii - half, ii] = 1.0      # rot[i] =  x[i-64], i >= 64
    return cosT, sinT, prot.astype(BF16NP)


def _in_maps(x, Wq, Wk, Wv, Wo):
    cosT, sinT, prot = _host_consts()
    # shared per-batch / per-group shards (read-only, safe to alias
    # across the in_maps of the 4 cores that use them)
    xTs = [np.ascontiguousarray(x[b].T.astype(BF16NP)) for b in range(B)]
    wqs = [np.ascontiguousarray(Wq[:, g * GD:(g + 1) * GD].astype(BF16NP))
           for g in range(G)]
    wks = [np.ascontiguousarray(Wk[:, g * HD:(g + 1) * HD].astype(BF16NP))
           for g in range(G)]
    wvs = [np.ascontiguousarray(Wv[:, g * HD:(g + 1) * HD].astype(BF16NP))
           for g in range(G)]
    wos = [np.ascontiguousarray(Wo[g * GD:(g + 1) * GD, :].astype(BF16NP))
           for g in range(G)]
    onesc = np.ones((P, 1), dtype=BF16NP)
    onesr = np.ones((1, P), dtype=np.float32)
    maps = []
    for c in range(8):
        b, g = c // 4, c % 4
        maps.append({
            "xT": xTs[b], "wq": wqs[g], "wk": wks[g], "wv": wvs[g],
            "wo": wos[g], "cosT": cosT, "sinT": sinT, "prot": prot,
            "onesc": onesc, "onesr": onesr,
        })
    return maps


def run(x, Wq, Wk, Wv, Wo, trace=False, **trace_kw):
    if "nc" not in _CACHE:
        _CACHE["nc"] = _build()
    nc = _CACHE["nc"]
    maps = _in_maps(
        np.asarray(x, dtype=np.float32), np.asarray(Wq, dtype=np.float32),
        np.asarray(Wk, dtype=np.float32), np.asarray(Wv, dtype=np.float32),
        np.asarray(Wo, dtype=np.float32))
    res = run_bass_kernel_spmd(
        nc, maps, core_ids=list(range(8)), trace=trace, **trace_kw)
    parts = [res.results[c]["out"] for c in range(8)]
    full = np.stack([
        parts[0] + parts[1] + parts[2] + parts[3],
        parts[4] + parts[5] + parts[6] + parts[7],
    ]).astype(np.float32)
    return full, res


def kernel(x, Wq, Wk, Wv, Wo, mask=None):
    full, _ = run(x, Wq, Wk, Wv, Wo, trace=False)
    return full


# revision 23
# speedup vs baseline: 1.0207x; 1.0207x over previous
"""GQA MultiHeadAttention (RoPE, causal) Bass/Tile kernel for 8 Trainium2 cores.

Problem: x[2,2048,2048] @ Wq/Wk/Wv -> RoPE -> causal GQA attention -> @ Wo.
D=2048, H=16 heads, G=4 KV groups, HD=128, B=2, S=2048.

Sharding (SPMD, one program, per-core data):
  core c -> batch b=c//4, KV-group g=c%4 (heads 4g..4g+3).
  Each core: QKV projection for its group from x[b]^T, RoPE, 4 heads of
  attention, and a row-shard of the output projection (Wo rows for its
  heads) producing a partial [2048,2048] output. Host sums the 4 partials
  per batch.

Design notes (vs v1 baseline at ~367us):
  - bf16 matmul path throughout (x, Wq/k/v/o, q/k, exp-weights, V, ctx);
    fp32 PSUM accumulation. Tolerance is 2e-2; this lands ~3.6e-3.
  - softmax denominator: no per-tile PE matmuls; exp tiles accumulate into
    an SBUF esum on DVE (bf16 2x), then ONE ones-vector matmul per
    (head, q-chunk) reduces over partitions.
  - causal trimming: diagonal k-tiles only compute scores/ctx/den on the
    q-column range at-or-below the diagonal; masks (widened to also clear
    the never-written columns) run on GpSimd.
  - k-tiles processed in PAIRS sharing a 2-bank PSUM tile so one ACT exp
    covers both (amortizes ~400ns/op ACT overhead). ACT does only exp in
    the attention phase; evictions are split across DVE/ACT.
  - DMA batching: descriptor issue costs ~0.6us on the Sync engine, so x
    and weights load in multi-d-tile groups, Wo is SBUF-resident, and the
    out-projection stores whole [128,2048] rows.
  - PSUM plan (8 banks): 2x 2-bank pair tiles (QKV q-accumulators /
    scores pairs) + 2 single banks (K/V accumulators / ctx + den-reduce)
    + 2 single banks (rope-rotation + V-transpose / broadcast + outproj).
  - rope/V-transpose tails interleave into the next s-chunk's projection
    stream; each q-chunk's out-projection interleaves into the next
    q-chunk's attention stream.
On-chip layouts all keep the feature dim on partitions: QT/KT [hd, s],
scoresT [k, q], ctxT [hd, q]; out-projection uses ctxT slices as stationary
to produce natural [s, d] partials.
"""

import sys

if "/opt/trn_rl_repo" not in sys.path:
    sys.path.insert(0, "/opt/trn_rl_repo")

from contextlib import ExitStack

import numpy as np
import ml_dtypes

import concourse.bass as bass
import concourse.tile as tile
from concourse import bacc, mybir
from concourse.bass_utils import run_bass_kernel_spmd
from concourse.masks import make_identity

F32 = mybir.dt.float32
F32R = mybir.dt.float32r
BF16 = mybir.dt.bfloat16
AF = mybir.ActivationFunctionType
BF16NP = ml_dtypes.bfloat16

B, S, D = 2, 2048, 2048
H, G, HD = 16, 4, 128
HPG = H // G          # heads per group = 4
GD = HPG * HD         # group width = 512
P = 128
NCHUNK = 512          # matmul moving free dim
SC = S // NCHUNK      # 4 s-chunks
DT = D // P           # 16 d-tiles
ST = S // P           # 16 s-tiles
SCALE = 1.0 / float(np.sqrt(HD))

_CACHE = {}


def _build():
    nc = bacc.Bacc("TRN2", target_bir_lowering=False, debug=False, num_devices=8)

    # ---- DRAM I/O (per-core shards) ----
    xT = nc.dram_tensor("xT", [D, S], BF16, kind="ExternalInput").ap()
    wq = nc.dram_tensor("wq", [D, GD], BF16, kind="ExternalInput").ap()
    wk = nc.dram_tensor("wk", [D, HD], BF16, kind="ExternalInput").ap()
    wv = nc.dram_tensor("wv", [D, HD], BF16, kind="ExternalInput").ap()
    wo = nc.dram_tensor("wo", [GD, D], BF16, kind="ExternalInput").ap()
    cosT = nc.dram_tensor("cosT", [HD, S], BF16, kind="ExternalInput").ap()
    sinT = nc.dram_tensor("sinT", [HD, S], BF16, kind="ExternalInput").ap()
    prot = nc.dram_tensor("prot", [HD, HD], BF16, kind="ExternalInput").ap()
    onesc = nc.dram_tensor("onesc", [P, 1], BF16, kind="ExternalInput").ap()
    onesr = nc.dram_tensor("onesr", [1, P], F32R, kind="ExternalInput").ap()
    out = nc.dram_tensor("out", [S, D], F32, kind="ExternalOutput").ap()

    xT_v = xT.rearrange("(t p) s -> p t s", p=P)          # [128, 16, 2048]
    wq_v = wq.rearrange("(t p) o -> p t o", p=P)          # [128, 16, 512]
    wk_v = wk.rearrange("(t p) o -> p t o", p=P)          # [128, 16, 128]
    wv_v = wv.rearrange("(t p) o -> p t o", p=P)
    wo_v = wo.rearrange("(h p) d -> p h d", p=P)          # [128, 4, 2048]
    out_v = out.rearrange("(t p) d -> t p d", p=P)        # [16, 128, 2048]

    with tile.TileContext(nc) as tc:
        with ExitStack() as ctx:
            pers = ctx.enter_context(tc.tile_pool(name="pers", bufs=1))
            psum = ctx.enter_context(tc.tile_pool(name="psum", bufs=2, space="PSUM"))
            rpsum = ctx.enter_context(tc.tile_pool(name="rpsum", bufs=2, space="PSUM"))
            xpool = ctx.enter_context(tc.tile_pool(name="xpool", bufs=2))
            x8pool = ctx.enter_context(tc.tile_pool(name="x8pool", bufs=3))
            spool = ctx.enter_context(tc.tile_pool(name="spool", bufs=3))
            epool = ctx.enter_context(tc.tile_pool(name="epool", bufs=5))
            dpool = ctx.enter_context(tc.tile_pool(name="dpool", bufs=3))
            cpool = ctx.enter_context(tc.tile_pool(name="cpool", bufs=2))
            evpool = ctx.enter_context(tc.tile_pool(name="evpool", bufs=2))
            opool = ctx.enter_context(tc.tile_pool(name="opool", bufs=2))

            _bank_n = [0]

            def bank():
                _bank_n[0] += 1
                return psum.tile([P, NCHUNK], F32, tag="bank",
                                 name=f"bank{_bank_n[0]}")

            def qpair():
                _bank_n[0] += 1
                return psum.tile([P, 2, NCHUNK], F32, tag="qpair",
                                 name=f"qpair{_bank_n[0]}")

            def rbank():
                _bank_n[0] += 1
                return rpsum.tile([P, NCHUNK], F32, tag="rbank",
                                  name=f"rbank{_bank_n[0]}")

            # ---- persistent tiles ----
            wq_t = pers.tile([P, DT, GD], BF16, tag="wq")
            wk_t = pers.tile([P, DT, HD], BF16, tag="wk")
            wv_t = pers.tile([P, DT, HD], BF16, tag="wv")
            cos_t = pers.tile([P, S], BF16, tag="cos")
            sin_t = pers.tile([P, S], BF16, tag="sin")
            prot_t = pers.tile([P, HD], BF16, tag="prot")
            ident = pers.tile([P, P], F32, tag="ident")
            qf = pers.tile([P, HPG, S], BF16, tag="qf")       # roped Q^T, 4 heads
            kf = pers.tile([P, S], BF16, tag="kf")            # roped K^T
            vnat = pers.tile([P, ST, HD], BF16, tag="vnat")   # V natural [s, hd]
            ones_col = pers.tile([P, 1], BF16, tag="ones_col")    # [K=128, M=1]
            ones_row = pers.tile([1, P], F32R, tag="ones_row")    # [K=1, M=128]
            wo_t = pers.tile([P, HPG, D], BF16, tag="wo")         # Wo rows, resident

            # Batched loads: DMA descriptor issue costs ~0.6us each on the
            # Sync queue, so group x/weight tiles into few large transfers.
            # sc0's x comes in 4-dt groups so the first matmuls start early.
            groups0 = [(0, 2), (2, 4), (4, 8), (8, 12), (12, 16)]
            x_tiles = {0: []}
            for lo, hi in groups0:
                nc.sync.dma_start(wq_t[:, lo:hi, :], wq_v[:, lo:hi, :])
                xg = xpool.tile([P, hi - lo, NCHUNK], BF16,
                                tag=f"xg{hi - lo}", name=f"xg0_{lo}")
                nc.sync.dma_start(xg[:], xT_v[:, lo:hi, 0:NCHUNK])
                x_tiles[0].append((lo, hi, xg))
            nc.sync.dma_start(wk_t[:], wk_v[:])
            nc.sync.dma_start(wv_t[:], wv_v[:])
            nc.sync.dma_start(cos_t[:], cosT[:])
            nc.sync.dma_start(sin_t[:], sinT[:])
            nc.sync.dma_start(prot_t[:], prot[:])
            nc.sync.dma_start(ones_col[:], onesc[:])
            nc.sync.dma_start(ones_row[:], onesr[:])
            make_identity(nc, ident[:])
            # PE warm-up: dummy transposes during the initial DMA wait keep
            # the HAM activity window busy so real matmuls start at 2.4GHz
            for _ in range(24):
                wb = rbank()
                nc.tensor.transpose(wb[:, 0:P], ident[:], ident[:])

            def fetch_x8(sc, g):
                xg = x8pool.tile([P, 8, NCHUNK], BF16, tag="xg8",
                                 name=f"xg{sc}_{g}")
                nc.sync.dma_start(
                    xg[:], xT_v[:, 8 * g:8 * (g + 1),
                                sc * NCHUNK:(sc + 1) * NCHUNK])
                return xg

            # ================= Phase A: QKV projection + RoPE + V^T -> V ====
            def rope(dst, src_sb, sc):
                """dst (bf16 slice [128,512]) = rope(src_sb bf16) for s-chunk sc."""
                cs = cos_t[:, sc * NCHUNK:(sc + 1) * NCHUNK]
                sn = sin_t[:, sc * NCHUNK:(sc + 1) * NCHUNK]
                rotps = rbank()
                nc.tensor.matmul(rotps[:], prot_t[:], src_sb, start=True, stop=True)
                t1 = spool.tile([P, NCHUNK], F32, tag="t1")
                nc.vector.tensor_mul(t1[:], rotps[:], sn)
                t2 = spool.tile([P, NCHUNK], F32, tag="t2")
                nc.vector.tensor_mul(t2[:], src_sb, cs)
                nc.vector.tensor_add(dst, t1[:], t2[:])

            def phase_a_proj(sc, dts):
                """QKV projection matmuls for s-chunk sc over d-tiles dts."""
                for dt in dts:
                    if sc == 0:
                        for lo, hi, xg in x_tiles[0]:
                            if lo <= dt < hi:
                                xt = xg[:, dt - lo, :]
                                break
                    else:
                        xt = x_tiles[sc][dt // 8][:, dt % 8, :]
                    st_flag = dt == 0
                    sp_flag = dt == DT - 1
                    qpr, kps, vps = _acc[sc]
                    for h in range(HPG):
                        nc.tensor.matmul(
                            qpr[h // 2][:, h % 2, :],
                            wq_t[:, dt, h * HD:(h + 1) * HD], xt,
                            start=st_flag, stop=sp_flag)
                    nc.tensor.matmul(kps[:], wk_t[:, dt, :], xt,
                                     start=st_flag, stop=sp_flag)
                    nc.tensor.matmul(vps[:], wv_t[:, dt, :], xt,
                                     start=st_flag, stop=sp_flag)
                    if sc + 1 < SC and dt in (6, 10):
                        x_tiles.setdefault(sc + 1, []).append(
                            fetch_x8(sc + 1, (dt - 6) // 4))

            def phase_a_evict(sc):
                """PSUM -> SBUF bf16 evictions (frees the 6 accumulator banks)."""
                qpr, kps, vps = _acc[sc]
                qsb4 = evpool.tile([P, HPG, NCHUNK], BF16, tag="ev_q",
                                   name=f"qsb{sc}")
                for h in range(HPG):
                    if h < 2:
                        nc.scalar.copy(qsb4[:, h, :], qpr[h // 2][:, h % 2, :])
                    else:
                        nc.vector.tensor_copy(qsb4[:, h, :],
                                              qpr[h // 2][:, h % 2, :])
                ksb = evpool.tile([P, NCHUNK], BF16, tag="ev_k", name=f"ksb{sc}")
                nc.vector.tensor_copy(ksb[:], kps[:])
                vsb = evpool.tile([P, NCHUNK], F32, tag="ev_v", name=f"vsb{sc}")
                nc.scalar.copy(vsb[:], vps[:])
                return qsb4, ksb, vsb

            def phase_a_tail_steps(sc, qsb4, ksb, vsb):
                """RoPE + V^T transpose (PE + DVE work, consumes evictions).
                Generator so the last s-chunk's tail can interleave with the
                first q-chunk's attention (fully data-independent)."""
                s0 = sc * NCHUNK
                for h in range(HPG):
                    rope(qf[:, h, s0:s0 + NCHUNK], qsb4[:, h, :], sc)
                    yield
                rope(kf[:, s0:s0 + NCHUNK], ksb[:], sc)
                yield
                tb = rbank()
                for j in range(4):
                    nc.tensor.transpose(
                        tb[:, j * P:(j + 1) * P], vsb[:, j * P:(j + 1) * P],
                        ident[:])
                yield
                nc.vector.tensor_copy(vnat[:, sc * 4:(sc + 1) * 4, :], tb[:])
                nc.sync.dma_start(wo_t[:, :, sc * NCHUNK:(sc + 1) * NCHUNK],
                                  wo_v[:, :, sc * NCHUNK:(sc + 1) * NCHUNK])
                yield

            def phase_a_tail(sc, qsb4, ksb, vsb):
                for _ in phase_a_tail_steps(sc, qsb4, ksb, vsb):
                    pass

            # ---- Phase B/C: attention per (q-chunk, head) + out-projection
            def den_engine(h):
                return nc.vector

            def norm_apply(ctxq, h, ctxps, denps):
                """reciprocal + partition-broadcast (K=1 matmul) + scale ctx."""
                rec32 = spool.tile([1, NCHUNK], F32, tag="rec32")
                nc.vector.reciprocal_approx_fast(rec32[:], denps[0:1, :])
                rec = spool.tile([1, NCHUNK], F32R, tag="rec")
                nc.vector.tensor_copy(rec[:], rec32[:])
                bps = rbank()
                nc.tensor.matmul(bps[:], ones_row[:], rec[:],
                                 start=True, stop=True)
                bsb = spool.tile([P, NCHUNK], F32, tag="bsb")
                nc.scalar.copy(bsb[:], bps[:])
                nc.vector.tensor_mul(ctxq[:, h, :], ctxps[:], bsb[:])

            LOOKAHEAD = 3   # in pairs (~6 k-tiles)

            def attention_steps(qc, ctxq):
                """Generator emitting attention for q-chunk qc one pipeline
                step at a time. k-tiles are processed in PAIRS sharing a
                2-bank PSUM tile so one ACT exp covers both (amortizes the
                ~400ns per-op ACT overhead). Scores matmuls stay causally
                trimmed; the diagonal masks are widened to also zero the
                never-written (garbage) PSUM columns of trimmed tiles."""
                q0 = qc * NCHUNK
                nki = 4 * qc + 4
                order = list(range(4 * qc, nki)) + list(range(4 * qc))
                npair = nki // 2
                pairs_seq = [(h, pi) for h in range(HPG) for pi in range(npair)]
                ets = {}
                esums = {}
                accs = {}
                pending = [None]

                def qlo_of(ki):
                    j = ki - 4 * qc
                    return P * j if j >= 0 else 0

                def do_scores(h, pi):
                    ki0, ki1 = order[2 * pi], order[2 * pi + 1]
                    sps = qpair()
                    for sl, ki in ((0, ki0), (1, ki1)):
                        qlo = qlo_of(ki)
                        nc.tensor.matmul(
                            sps[:, sl, qlo:], kf[:, ki * P:(ki + 1) * P],
                            qf[:, h, q0 + qlo:q0 + NCHUNK],
                            start=True, stop=True)
                    et2 = epool.tile([P, 2, NCHUNK], BF16, tag="et2",
                                     name=f"et{qc}_{h}_{pi}")
                    nc.scalar.activation(et2[:], sps[:], AF.Exp, scale=SCALE)
                    for sl, ki in ((0, ki0), (1, ki1)):
                        j = ki - 4 * qc
                        if j >= 0:
                            # zero below-diagonal AND the garbage columns
                            # left of the trimmed range: cols [0, 128(j+1))
                            nc.gpsimd.affine_select(
                                out=et2[:, sl, 0:P * (j + 1)],
                                in_=et2[:, sl, 0:P * (j + 1)],
                                compare_op=mybir.AluOpType.is_ge,
                                fill=0.0,
                                base=-(P * j),
                                channel_multiplier=-1,
                                pattern=[[1, P * (j + 1)]],
                            )
                    for sl, ki in ((0, ki0), (1, ki1)):
                        qlo = qlo_of(ki)
                        if pi == 0 and sl == 0:
                            esum = dpool.tile([P, NCHUNK], BF16, tag="esum",
                                              name=f"esum{qc}_{h}")
                            esums[h] = esum
                            nc.vector.tensor_copy(esum[:], et2[:, 0, :])
                        else:
                            esum = esums[h]
                            nc.vector.tensor_add(esum[:, qlo:], esum[:, qlo:],
                                                 et2[:, sl, qlo:])
                    ets[(h, pi)] = et2

                def do_ctx(h, pi):
                    if h not in accs:
                        accs[h] = bank()
                        if pending[0] is not None:
                            norm_apply(ctxq, *pending[0])
                            pending[0] = None
                    ctxps = accs[h]
                    ki0, ki1 = order[2 * pi], order[2 * pi + 1]
                    et2 = ets.pop((h, pi))
                    for sl, ki in ((0, ki0), (1, ki1)):
                        qlo = qlo_of(ki)
                        nc.tensor.matmul(
                            ctxps[:, qlo:], vnat[:, ki, :], et2[:, sl, qlo:],
                            start=(pi == 0 and sl == 0),
                            stop=(pi == npair - 1 and sl == 1))
                    if pi == npair - 1:
                        # head finished: one partition-reduce matmul for the
                        # denominator; recip+broadcast+apply deferred into
                        # the next head's stream
                        denps = rbank()
                        nc.tensor.matmul(denps[0:1, :], ones_col[:],
                                         esums.pop(h)[:], start=True,
                                         stop=True)
                        pending[0] = (h, ctxps, denps)
                        del accs[h]

                for i, (h, pi) in enumerate(pairs_seq):
                    do_scores(h, pi)
                    if i >= LOOKAHEAD:
                        do_ctx(*pairs_seq[i - LOOKAHEAD])
                    yield
                for i in range(len(pairs_seq) - LOOKAHEAD, len(pairs_seq)):
                    do_ctx(*pairs_seq[i])
                    yield
                norm_apply(ctxq, *pending[0])

            def outproj_steps(qc, ctxq):
                """Generator emitting the out-projection for q-chunk qc one
                (dc, st) group at a time; interleaved into the NEXT q-chunk's
                attention stream so its Wo DMAs and eviction chains hide."""
                for st in range(4):
                    stq = qc * 4 + st
                    osb = opool.tile([P, SC, NCHUNK], F32, tag="osb",
                                     name=f"osb{qc}_{st}")
                    for dc in range(SC):
                        ops = rbank()
                        for h in range(HPG):
                            nc.tensor.matmul(
                                ops[:], ctxq[:, h, st * P:(st + 1) * P],
                                wo_t[:, h, dc * NCHUNK:(dc + 1) * NCHUNK],
                                start=(h == 0), stop=(h == HPG - 1))
                        if dc % 2 == 0:
                            nc.vector.tensor_copy(osb[:, dc, :], ops[:])
                        else:
                            nc.scalar.copy(osb[:, dc, :], ops[:])
                        yield
                    nc.sync.dma_start(out_v[stq, :, :], osb[:])

            # Interleave: s-chunk tails (rope/transpose, PE+DVE) are emitted
            # after the next s-chunk's first projection matmuls so the PE
            # queue never drains while evictions/ropes complete.
            _acc = {}
            tail_gen = None
            for sc in range(SC):
                _acc[sc] = ([qpair(), qpair()], bank(), bank())
                for dt in range(DT):
                    phase_a_proj(sc, [dt])
                    if tail_gen is not None and dt >= 5 and dt % 2 == 1:
                        next(tail_gen, None)
                if tail_gen is not None:
                    for _ in tail_gen:
                        pass
                tail_gen = phase_a_tail_steps(sc, *phase_a_evict(sc))

            # Phase B: each q-chunk's attention, with the PREVIOUS q-chunk's
            # out-projection groups spread through it. The final s-chunk's
            # rope/transpose tail seeds the interleave stream for q-chunk 0.
            out_gen = tail_gen
            for qc in range(SC):
                ctxq = cpool.tile([P, HPG, NCHUNK], BF16, tag="ctxq",
                                  name=f"ctxq{qc}")
                n_steps = 4 * (2 * qc + 2) + LOOKAHEAD
                ratio = max(1, n_steps // 17)
                k = 0
                delay = 6 if qc == 0 else 0
                for _ in attention_steps(qc, ctxq):
                    k += 1
                    if out_gen is not None and k >= delay and k % ratio == 0:
                        next(out_gen, None)
                if out_gen is not None:
                    for _ in out_gen:
                        pass
                out_gen = outproj_steps(qc, ctxq)
            for _ in out_gen:
                pass

    nc.compile()
    return nc


def _host_consts():
    i = np.arange(0, HD, 2, dtype=np.float32)
    inv = (1.0 / (10000.0 ** (i / HD))).astype(np.float32)      # [64]
    t = np.arange(S, dtype=np.float32)
    freqs = t[:, None] * inv[None, :]                           # [S, 64] f32
    emb = np.concatenate([freqs, freqs], axis=1)                # [S, 128]
    cosT = np.ascontiguousarray(np.cos(emb).T.astype(BF16NP))   # [128, S]
    sinT = np.ascontiguousarray(np.sin(emb).T.astype(BF16NP))
    prot = np.zeros((HD, HD), dtype=np.float32)
    half = HD // 2
    for ii in range(half):
        prot[ii + half, ii] = -1.0     # rot[i] = -x[i+64], i < 64
    for ii in range(half, HD):
        prot[ii - half, ii] = 1.0      # rot[i] =  x[i-64], i >= 64
    return cosT, sinT, prot.astype(BF16NP)


def _in_maps(x, Wq, Wk, Wv, Wo):
    cosT, sinT, prot = _host_consts()
    # shared per-batch / per-group shards (read-only, safe to alias
    # across the in_maps of the 4 cores that use them)
    xTs = [np.ascontiguousarray(x[b].T.astype(BF16NP)) for b in range(B)]
    wqs = [np.ascontiguousarray(Wq[:, g * GD:(g + 1) * GD].astype(BF16NP))
           for g in range(G)]
    wks = [np.ascontiguousarray(Wk[:, g * HD:(g + 1) * HD].astype(BF16NP))
           for g in range(G)]
    wvs = [np.ascontiguousarray(Wv[:, g * HD:(g + 1) * HD].astype(BF16NP))
           for g in range(G)]
    wos = [np.ascontiguousarray(Wo[g * GD:(g + 1) * GD, :].astype(BF16NP))
           for g in range(G)]
    onesc = np.ones((P, 1), dtype=BF16NP)
    onesr = np.ones((1, P), dtype=np.float32)
    maps = []
    for c in range(8):
        b, g = c // 4, c % 4
        maps.append({
            "xT": xTs[b], "wq": wqs[g], "wk": wks[g], "wv": wvs[g],
            "wo": wos[g], "cosT": cosT, "sinT": sinT, "prot": prot,
            "onesc": onesc, "onesr": onesr,
        })
    return maps


def run(x, Wq, Wk, Wv, Wo, trace=False, **trace_kw):
    if "nc" not in _CACHE:
        _CACHE["nc"] = _build()
    nc = _CACHE["nc"]
    maps = _in_maps(
        np.asarray(x, dtype=np.float32), np.asarray(Wq, dtype=np.float32),
        np.asarray(Wk, dtype=np.float32), np.asarray(Wv, dtype=np.float32),
        np.asarray(Wo, dtype=np.float32))
    res = run_bass_kernel_spmd(
        nc, maps, core_ids=list(range(8)), trace=trace, **trace_kw)
    parts = [res.results[c]["out"] for c in range(8)]
    full = np.stack([
        parts[0] + parts[1] + parts[2] + parts[3],
        parts[4] + parts[5] + parts[6] + parts[7],
    ]).astype(np.float32)
    return full, res


def kernel(x, Wq, Wk, Wv, Wo, mask=None):
    full, _ = run(x, Wq, Wk, Wv, Wo, trace=False)
    return full
